# revision 1
# baseline (speedup 1.0000x reference)
"""ArcFace inner-product kernel for one TRN2 chip (8 NeuronCores).

Problem: feat [4096, 512] f32, label [4096] i64, weights [20000, 512] f32.
  nf = l2norm(feat, axis=1); nw = l2norm(weights, axis=1)
  cos = nf @ nw.T                               [4096, 20000]
  ml  = 30 * cos(arccos(cos) + margin-at-label) [4096, 20000]
Returns (cos, ml).

Sharding: tensor-parallel over the class dim C. Each core receives the
full feat plus a 2500-row slice of weights and produces the matching
2500-column slices of both outputs. No collectives: the per-row label
fixup touches only 4096 of the 82M output elements and is applied on the
host after the column-slice gather.

Final design (measured ~192us/body on HW vs the 543us f32 baseline;
TimelineSim cost model predicts 188us; rel_err 3.7e-3 vs the f32
reference, well under the 2e-2 gate):
  - bf16 everywhere the error budget allows: matmul operands (nfT/nwT)
    and BOTH outputs are bf16 (host upcasts to f32). Output HBM traffic
    halves: 82MB -> 41MB per core, which was the f32 roofline.
  - inputs are cast f32->bf16 during the load itself (SWDGE gpsimd
    DMA), batched 2 tiles per DMA (SWDGE fixed cost ~1us dominates
    small transfers), leaving both HWDGE rings to the output strips.
  - output DMAs batched 2 m-strips per DMA (1.28MB bf16, sync ring:
    cos, scalar ring: ml).
  - k-outer matmul order with one live PSUM bank per n-chunk (7 mm
    banks + 1 transpose bank).
  - evicts: ACT evicts cos chunks n=0..3 from PSUM (per-partition
    scale rf); DVE evicts the short n=4 chunk; ml is ONE 2500-wide
    bf16 tensor_scalar_mul per m-strip read from the bf16 cos strip
    (all-16-bit SBUF operands hit DVE's 2x/4x mode).
  - feat prep interleaved into weight prep so the main loop starts
    without a pipeline refill lull; feat prefetch distance 3 pairs.

"""

import math

import numpy as np

from concourse import bacc, mybir, tile
from concourse.masks import make_identity
from concourse.bass_utils import run_bass_kernel_spmd

B, D, C = 4096, 512, 20000
NCORES = 8
CLOC = C // NCORES  # 2500
KCH = D // 128      # 4 k-chunks
MT = B // 128       # 32 B-tiles
NT = (CLOC + 511) // 512  # 5 n-chunks (last = 452)
WT = (CLOC + 127) // 128  # 20 w-tiles (last = 68 rows)
SPD = 2             # m-strips batched per output DMA
GROUPS = MT // SPD  # 8 output DMA groups per output tensor
PF = 6              # feat-tile prefetch distance

SCALE = 30.0
MARGIN = 0.5
THRESH = -math.cos(MARGIN)
EXT_VAL = -MARGIN * math.sin(MARGIN)
COS_M = math.cos(MARGIN)
SIN_M = math.sin(MARGIN)

F32 = mybir.dt.float32
BF16 = mybir.dt.bfloat16

_NC_CACHE = {}


def _build_nc(repeats=1, timing=False):
    # timing=True: big outputs become Internal DRAM scratch (same DMA
    # traffic, nothing shipped over the axon tunnel per call) + a tiny
    # token ExternalOutput, fenced by reading the outputs back.
    nc = bacc.Bacc(
        "TRN2",
        target_bir_lowering=False,
        debug=False,
        num_devices=NCORES,
    )
    out_kind = "Internal" if timing else "ExternalOutput"
    feat = nc.dram_tensor("feat", [B, D], F32, kind="ExternalInput").ap()
    w = nc.dram_tensor("w", [CLOC, D], F32, kind="ExternalInput").ap()
    cos_o = nc.dram_tensor("cos_o", [B, CLOC], BF16, kind=out_kind).ap()
    ml_o = nc.dram_tensor("ml_o", [B, CLOC], BF16, kind=out_kind).ap()
    tok_o = (
        nc.dram_tensor("tok_o", [128, 4], F32, kind="ExternalOutput").ap()
        if timing
        else None
    )

    with tile.TileContext(nc) as tc:
        with (
            tc.tile_pool(name="const", bufs=1) as const_pool,
            tc.tile_pool(name="persist", bufs=1) as persist,
            tc.tile_pool(name="stage", bufs=6) as stage,
            tc.tile_pool(name="scratch", bufs=4) as scratch,
            tc.tile_pool(name="outs", bufs=3) as outs,
            tc.tile_pool(name="mm_psum", bufs=7, space="PSUM") as mm_psum,
            tc.tile_pool(name="tp_psum", bufs=1, space="PSUM") as tp_psum,
        ):
            ident = const_pool.tile([128, 128], BF16, tag="ident")
            make_identity(nc, ident[:])
            NSZ = [min(512, CLOC - n * 512) for n in range(NT)]

            def body(rep):
                sfx = f"_r{rep}" if rep else ""
                nfT = [
                    persist.tile([128, KCH * 128], BF16, tag=f"nfT{m}",
                                 name=f"nfT{m}{sfx}")
                    for m in range(MT)
                ]
                nwT = [
                    persist.tile([128, KCH * NSZ[n]], BF16, tag=f"nwT{n}",
                                 name=f"nwT{n}{sfx}")
                    for n in range(NT)
                ]
                rf = [
                    persist.tile([128, 1], F32, tag=f"rf{m}", name=f"rf{m}{sfx}")
                    for m in range(MT)
                ]
                emit(rep, nfT, nwT, rf)

            def rnorm(xt, rows, r_out):
                """r_out[p] = 1/||xt[p,:]|| for the first `rows` partitions.
                (ACT Square+accum_out; vector.tensor_tensor_reduce faults the
                hardware in this environment - do not use it.)"""
                sq = scratch.tile([128, D], F32, tag="sq")
                n2 = scratch.tile([128, 1], F32, tag="n2")
                nc.scalar.activation(
                    sq[:rows],
                    xt[:rows],
                    mybir.ActivationFunctionType.Square,
                    accum_out=n2[:rows],
                )
                rinv = scratch.tile([128, 1], F32, tag="rinv")
                nc.vector.reciprocal(rinv[:rows], n2[:rows])
                nc.scalar.sqrt(r_out[:rows], rinv[:rows])

            def transpose_blocks(xt, rows):
                """Transpose the 4 [rows,128] blocks of xt into one PSUM bank
                laid out k-major; returns the [128, KCH*128] PSUM tile."""
                tp = tp_psum.tile([128, KCH * 128], BF16, tag="tp")
                for k in range(KCH):
                    nc.tensor.transpose(
                        tp[:, k * 128 : k * 128 + rows],
                        xt[:rows, k * 128 : (k + 1) * 128],
                        ident[:rows, :rows],
                    )
                return tp

            def emit(rep, nfT, nwT, rf):
                sfx = f"_r{rep}" if rep else ""

                def wprep_tile(xt, t, rows):
                    """xt: [128(rows), D] bf16 view holding w tile t."""
                    n = t // 4
                    off = (t % 4) * 128
                    rw = scratch.tile([128, 1], F32, tag="rw")
                    rnorm(xt, rows, rw)
                    nc.vector.tensor_scalar_mul(xt[:rows], xt[:rows], rw[:rows])
                    tp = transpose_blocks(xt, rows)
                    src = tp[:].rearrange("p (k c) -> p k c", k=KCH)[:, :, :rows]
                    dst = nwT[n][:].rearrange("p (k c) -> p k c", k=KCH)[
                        :, :, off : off + rows
                    ]
                    nc.vector.tensor_copy(dst, src)

                def w_pair(p):
                    """Paired cast-loads (SWDGE fixed cost ~1us per DMA
                    dominates small loads, so batch 2 tiles per DMA)."""
                    xt2 = stage.tile([128, 2 * D], BF16, tag="xt2",
                                     name=f"xw{p}{sfx}")
                    nc.gpsimd.dma_start(
                        out=xt2[:].rearrange("p (s c) -> p s c", s=2),
                        in_=w[2 * p * 128 : (2 * p + 2) * 128, :].rearrange(
                            "(s p) c -> p s c", s=2
                        ),
                    )
                    wprep_tile(xt2[:, :D], 2 * p, 128)
                    wprep_tile(xt2[:, D:], 2 * p + 1, 128)

                def w_tail(t):  # ragged tail: 128 + 68 rows
                    rows = min(128, CLOC - t * 128)
                    xt2 = stage.tile([128, 2 * D], BF16, tag="xt2",
                                     name=f"xw_t{t}{sfx}")
                    nc.gpsimd.dma_start(
                        out=xt2[:rows, :D], in_=w[t * 128 : t * 128 + rows, :]
                    )
                    wprep_tile(xt2[:, :D], t, rows)

                # Feat prep: paired cast-loads; row norm saved per m-tile.
                def feat_pair(q):
                    xt2 = stage.tile([128, 2 * D], BF16, tag="xt2",
                                     name=f"xf{q}{sfx}")
                    nc.gpsimd.dma_start(
                        out=xt2[:].rearrange("p (s c) -> p s c", s=2),
                        in_=feat[2 * q * 128 : (2 * q + 2) * 128, :].rearrange(
                            "(s p) c -> p s c", s=2
                        ),
                    )
                    for b in (0, 1):
                        t = 2 * q + b
                        xt = xt2[:, b * D : (b + 1) * D]
                        rnorm(xt, 128, rf[t])
                        tp = transpose_blocks(xt, 128)
                        nc.vector.tensor_copy(nfT[t][:], tp[:])

                # Interleave the prefetched feat pairs into the w-prep
                # stream so both finish together and the main loop starts
                # without a pipeline refill lull.
                PFQ = 3  # feat prefetch distance, in pairs
                for p in range(WT // 2 - 1):
                    w_pair(p)
                    if p in (2, 4, 6):
                        feat_pair((p - 2) // 2)
                w_tail(WT - 2)
                w_tail(WT - 1)

                for g in range(GROUPS):
                    cos_strip = outs.tile([128, SPD * CLOC], BF16,
                                          tag="cos_strip")
                    ml_strip = outs.tile([128, SPD * CLOC], BF16,
                                         tag="ml_strip")
                    for j in range(SPD):
                        m = g * SPD + j
                        if m % 2 == 0 and m // 2 + PFQ < MT // 2:
                            feat_pair(m // 2 + PFQ)
                        # k-outer: the stationary lhsT block is loaded once
                        # per (m, k) and reused across the 5 n-chunks
                        # (ldweights=False on the reuse matmuls), cutting
                        # PE-SEQ Ldweights dispatches 5x. Requires one live
                        # PSUM bank per n-chunk.
                        pss = [
                            mm_psum.tile([128, 512], F32, tag="mm",
                                         name=f"mm{m}_{n}{sfx}")
                            for n in range(NT)
                        ]
                        for k in range(KCH):
                            for n in range(NT):
                                nsz = NSZ[n]
                                inst = nc.tensor.matmul(
                                    pss[n][:, :nsz],
                                    lhsT=nfT[m][:, k * 128 : (k + 1) * 128],
                                    rhs=nwT[n][:, k * nsz : (k + 1) * nsz],
                                    start=(k == 0),
                                    stop=(k == KCH - 1),
                                )
                                if n > 0:
                                    inst.ldweights = False
                        # cos evicts: ACT n=0..3, DVE the short n=4 chunk
                        # (balances ACT ~130us vs DVE ~115us per body).
                        for n in range(NT):
                            nsz = NSZ[n]
                            c0 = j * CLOC + n * 512
                            if n < NT - 1:
                                nc.scalar.activation(
                                    cos_strip[:, c0 : c0 + nsz],
                                    pss[n][:, :nsz],
                                    mybir.ActivationFunctionType.Copy,
                                    scale=rf[m][:],
                                )
                            else:
                                nc.vector.tensor_scalar_mul(
                                    cos_strip[:, c0 : c0 + nsz],
                                    pss[n][:, :nsz],
                                    rf[m][:],
                                )
                        # ml = 30*cos read from the bf16 strip: all-bf16
                        # SBUF operands put DVE in its 2x/4x 16-bit mode,
                        # and it is one instruction per m instead of five.
                        nc.vector.tensor_scalar_mul(
                            ml_strip[:, j * CLOC : (j + 1) * CLOC],
                            cos_strip[:, j * CLOC : (j + 1) * CLOC],
                            SCALE,
                        )
                    r0 = g * SPD * 128
                    nc.sync.dma_start(
                        out=cos_o[r0 : r0 + SPD * 128, :].rearrange(
                            "(s p) c -> p s c", s=SPD
                        ),
                        in_=cos_strip[:].rearrange("p (s c) -> p s c", s=SPD),
                    )
                    nc.scalar.dma_start(
                        out=ml_o[r0 : r0 + SPD * 128, :].rearrange(
                            "(s p) c -> p s c", s=SPD
                        ),
                        in_=ml_strip[:].rearrange("p (s c) -> p s c", s=SPD),
                    )

            for rep in range(repeats):
                body(rep)

            if timing:
                # Fence: read back a sliver of each Internal output on both
                # output rings (FIFO per ring), keeping writes live vs
                # dead-store elimination and gating the token on the drain.
                tok = const_pool.tile([128, 4], F32, tag="tok")
                tokb = const_pool.tile([128, 4], BF16, tag="tokb")
                nc.sync.dma_start(out=tokb[:, :2], in_=cos_o[B - 128 :, :2])
                nc.scalar.dma_start(out=tokb[:, 2:4], in_=ml_o[B - 128 :, :2])
                nc.vector.tensor_copy(tok[:], tokb[:])
                nc.sync.dma_start(out=tok_o, in_=tok[:])

    nc.compile()
    return nc


def _purge_neff_cache():
    """The neuronxcc NEFF cache key does NOT cover the embedded BIR
    payload (verified: edited kernels cache-hit stale NEFFs compiled
    from different BIR). Purge it so this process always executes the
    NEFF compiled from THIS module."""
    import shutil

    shutil.rmtree("/root/.neuron-compile-cache", ignore_errors=True)


def _get_nc():
    if "nc" not in _NC_CACHE:
        _purge_neff_cache()
        _NC_CACHE["nc"] = _build_nc()
    return _NC_CACHE["nc"]


def make_in_maps(feat, weights):
    feat = np.ascontiguousarray(np.asarray(feat, dtype=np.float32))
    weights = np.ascontiguousarray(np.asarray(weights, dtype=np.float32))
    return [
        {"feat": feat, "w": weights[k * CLOC : (k + 1) * CLOC]}
        for k in range(NCORES)
    ]


def assemble(results, label):
    """Gather per-core column slices (bf16 -> f32) and apply the per-row
    label fixup."""
    cos = np.empty((B, C), np.float32)
    ml = np.empty((B, C), np.float32)
    for k in range(NCORES):
        cos[:, k * CLOC : (k + 1) * CLOC] = results[k]["cos_o"].astype(
            np.float32
        )
        ml[:, k * CLOC : (k + 1) * CLOC] = results[k]["ml_o"].astype(
            np.float32
        )
    idx = np.arange(B)
    lab = np.asarray(label).astype(np.int64)
    cil = cos[idx, lab]
    sin_il = np.sqrt(np.maximum(0.0, 1.0 - cil * cil)).astype(np.float32)
    hit = cil > THRESH
    ml[idx, lab] = np.where(
        hit,
        SCALE * (cil * COS_M - sin_il * SIN_M),
        SCALE * (cil + EXT_VAL),
    ).astype(np.float32)
    return cos, ml


def kernel(feat, label, weights):
    nc = _get_nc()
    in_maps = make_in_maps(feat, weights)
    res = run_bass_kernel_spmd(nc, in_maps, core_ids=list(range(NCORES)))
    return assemble(res.results, label)



# revision 7
# speedup vs baseline: 498.1484x; 498.1484x over previous
"""ArcFace inner-product kernel for one TRN2 chip (8 NeuronCores).

Problem: feat [4096, 512] f32, label [4096] i64, weights [20000, 512] f32.
  nf = l2norm(feat, axis=1); nw = l2norm(weights, axis=1)
  cos = nf @ nw.T                               [4096, 20000]
  ml  = 30 * cos(arccos(cos) + margin-at-label) [4096, 20000]
Returns (cos, ml).

Sharding: tensor-parallel over the class dim C. Each core receives the
full feat plus a 2500-row slice of weights and produces the matching
2500-column slices of both outputs. No collectives: the per-row label
fixup touches only 4096 of the 82M output elements and is applied on the
host after the column-slice gather.

Design (v2, from TimelineSim gap analysis of the 192.8us v1):
  - bf16 everywhere the error budget allows: matmul operands (nfT/nwT)
    and BOTH outputs are bf16 (host upcasts to f32). Output HBM traffic
    halves vs f32: 41MB per core.
  - inputs cast f32->bf16 during the load itself (SWDGE gpsimd DMA),
    batched FOUR 128-row tiles per DMA (SWDGE fixed cost ~1us dominates),
    leaving the HWDGE rings to the output strips.
  - k-outer matmul order with one live PSUM bank per n-chunk (6 mm
    banks + 2 transpose banks). v1 used 7+1; the single transpose bank
    made PE wait ~0.73us on the DVE drain after every 4-transpose feat
    block (~23us/body). Two tp banks let PE ping-pong.
  - with only 6 mm banks, chunk n of row m+1 reuses the bank of chunk
    n-1 of row m, so evictions must keep pace with PE's ~0.21us/chunk
    k0 row. One engine can't (0.6us/evict): ACT evicts n=0..2, DVE
    evicts n=3..4. ml is ONE 2500-wide bf16 tensor_scalar_mul per
    m-strip read from the bf16 cos strip (all-16-bit operands hit DVE's
    2x mode).
  - outputs are written per m-strip (SPD=1): cos on the sync(SP) ring,
    ml on the scalar(ACT) ring (the only two HWDGE rings), draining
    the tail ~10us sooner than v1's per-group strips.
  - feat pipeline: one tile per m processed 8 tiles ahead; quads 0-1
    interleave into the w-prep stream so the main loop starts without a
    refill lull.
"""

import math

import numpy as np

from concourse import bacc, mybir, tile
from concourse.masks import make_identity
from concourse.bass_utils import run_bass_kernel_spmd

B, D, C = 4096, 512, 20000
NCORES = 8
CLOC = C // NCORES  # 2500
KCH = D // 128      # 4 k-chunks
MT = B // 128       # 32 B-tiles
NT = (CLOC + 511) // 512  # 5 n-chunks (last = 452)
WT = (CLOC + 127) // 128  # 20 w-tiles (last = 68 rows)
FT_AHEAD = 8        # feat tile prefetch distance, in tiles

SCALE = 30.0
MARGIN = 0.5
THRESH = -math.cos(MARGIN)
EXT_VAL = -MARGIN * math.sin(MARGIN)
COS_M = math.cos(MARGIN)
SIN_M = math.sin(MARGIN)

F32 = mybir.dt.float32
BF16 = mybir.dt.bfloat16

_NC_CACHE = {}


def _build_nc(repeats=1, timing=False):
    # timing=True: big outputs become Internal DRAM scratch (same DMA
    # traffic, nothing shipped over the axon tunnel per call) + a tiny
    # token ExternalOutput, fenced by reading the outputs back.
    nc = bacc.Bacc(
        "TRN2",
        target_bir_lowering=False,
        debug=False,
        num_devices=NCORES,
    )
    out_kind = "Internal" if timing else "ExternalOutput"
    feat = nc.dram_tensor("feat", [B, D], F32, kind="ExternalInput").ap()
    w = nc.dram_tensor("w", [CLOC, D], F32, kind="ExternalInput").ap()
    cos_o = nc.dram_tensor("cos_o", [B, CLOC], BF16, kind=out_kind).ap()
    ml_o = nc.dram_tensor("ml_o", [B, CLOC], BF16, kind=out_kind).ap()
    tok_o = (
        nc.dram_tensor("tok_o", [128, 4], F32, kind="ExternalOutput").ap()
        if timing
        else None
    )

    with tile.TileContext(nc) as tc:
        with (
            tc.tile_pool(name="const", bufs=1) as const_pool,
            tc.tile_pool(name="persist", bufs=1) as persist,
            tc.tile_pool(name="stage", bufs=5) as stage,
            tc.tile_pool(name="scratch", bufs=4) as scratch,
            tc.tile_pool(name="outs", bufs=3) as outs,
            tc.tile_pool(name="mm_psum", bufs=6, space="PSUM") as mm_psum,
            tc.tile_pool(name="tp_psum", bufs=2, space="PSUM") as tp_psum,
        ):
            ident = const_pool.tile([128, 128], BF16, tag="ident")
            make_identity(nc, ident[:])
            NSZ = [min(512, CLOC - n * 512) for n in range(NT)]

            def body(rep):
                sfx = f"_r{rep}" if rep else ""
                nfT = [
                    persist.tile([128, KCH * 128], BF16, tag=f"nfT{m}",
                                 name=f"nfT{m}{sfx}")
                    for m in range(MT)
                ]
                nwT = [
                    persist.tile([128, KCH * NSZ[n]], BF16, tag=f"nwT{n}",
                                 name=f"nwT{n}{sfx}")
                    for n in range(NT)
                ]
                rf = [
                    persist.tile([128, 1], F32, tag=f"rf{m}", name=f"rf{m}{sfx}")
                    for m in range(MT)
                ]
                emit(rep, nfT, nwT, rf)

            def rnorm(xt, rows, r_out):
                """r_out[p] = 1/||xt[p,:]|| for the first `rows` partitions.
                (ACT Square+accum_out; vector.tensor_tensor_reduce faults the
                hardware in this environment - do not use it.)"""
                sq = scratch.tile([128, D], F32, tag="sq")
                n2 = scratch.tile([128, 1], F32, tag="n2")
                nc.scalar.activation(
                    sq[:rows],
                    xt[:rows],
                    mybir.ActivationFunctionType.Square,
                    accum_out=n2[:rows],
                )
                rinv = scratch.tile([128, 1], F32, tag="rinv")
                nc.vector.reciprocal(rinv[:rows], n2[:rows])
                nc.scalar.sqrt(r_out[:rows], rinv[:rows])

            def transpose_blocks(xt, rows):
                """Transpose the 4 [rows,128] blocks of xt into one PSUM bank
                laid out k-major; returns the [128, KCH*128] PSUM tile."""
                tp = tp_psum.tile([128, KCH * 128], BF16, tag="tp")
                for k in range(KCH):
                    nc.tensor.transpose(
                        tp[:, k * 128 : k * 128 + rows],
                        xt[:rows, k * 128 : (k + 1) * 128],
                        ident[:rows, :rows],
                    )
                return tp

            def emit(rep, nfT, nwT, rf):
                sfx = f"_r{rep}" if rep else ""

                def wprep_tile(xt, t, rows):
                    """xt: [128(rows), D] bf16 view holding w tile t."""
                    n = t // 4
                    off = (t % 4) * 128
                    rw = scratch.tile([128, 1], F32, tag="rw")
                    rnorm(xt, rows, rw)
                    nc.vector.tensor_scalar_mul(xt[:rows], xt[:rows], rw[:rows])
                    tp = transpose_blocks(xt, rows)
                    src = tp[:].rearrange("p (k c) -> p k c", k=KCH)[:, :, :rows]
                    dst = nwT[n][:].rearrange("p (k c) -> p k c", k=KCH)[
                        :, :, off : off + rows
                    ]
                    nc.vector.tensor_copy(dst, src)

                # SWDGE cast-loads: fixed cost ~1us per DMA dominates small
                # transfers, so batch 4 tiles per DMA. The first w load is a
                # small pair so prep starts sooner.
                def w_load(t0, nt, name):
                    """Load w tiles t0..t0+nt-1 (full 128-row tiles) into one
                    stage tile; returns (stage_tile, views)."""
                    xt4 = stage.tile([128, 4 * D], BF16, tag="xt4", name=name)
                    nc.gpsimd.dma_start(
                        out=xt4[:, : nt * D].rearrange("p (s c) -> p s c", s=nt),
                        in_=w[t0 * 128 : (t0 + nt) * 128, :].rearrange(
                            "(s p) c -> p s c", s=nt
                        ),
                    )
                    return xt4

                def feat_quad_load(q):
                    xt4 = stage.tile([128, 4 * D], BF16, tag="fq",
                                     name=f"xf{q}{sfx}")
                    nc.gpsimd.dma_start(
                        out=xt4[:].rearrange("p (s c) -> p s c", s=4),
                        in_=feat[4 * q * 128 : (4 * q + 4) * 128, :].rearrange(
                            "(s p) c -> p s c", s=4
                        ),
                    )
                    return xt4

                def feat_prep_tile(xt, t):
                    rnorm(xt, 128, rf[t])
                    tp = transpose_blocks(xt, 128)
                    nc.vector.tensor_copy(nfT[t][:], tp[:])

                fq_tiles = {}  # quad index -> stage tile
                strips = {}  # m -> (cos_strip, ml_strip)
                M0 = 3  # rows processed chunk-by-chunk during prep

                def new_strips(m):
                    strips[m] = (
                        outs.tile([128, CLOC], BF16, tag="cos_strip",
                                  name=f"cosst{m}{sfx}"),
                        outs.tile([128, CLOC], BF16, tag="ml_strip",
                                  name=f"mlst{m}{sfx}"),
                    )

                def evict(m, n, ps):
                    nsz = NSZ[n]
                    c0 = n * 512
                    if n < 3:
                        nc.scalar.activation(
                            strips[m][0][:, c0 : c0 + nsz],
                            ps[:, :nsz],
                            mybir.ActivationFunctionType.Copy,
                            scale=rf[m][:],
                        )
                    else:
                        nc.vector.tensor_scalar_mul(
                            strips[m][0][:, c0 : c0 + nsz],
                            ps[:, :nsz],
                            rf[m][:],
                        )

                def finish_m(m):
                    cos_strip, ml_strip = strips[m]
                    # ml = 30*cos read from the bf16 strip: all-bf16 operands
                    # put DVE in its 2x 16-bit mode; one instruction per m.
                    nc.vector.tensor_scalar_mul(ml_strip[:], cos_strip[:], SCALE)
                    r0 = m * 128
                    nc.sync.dma_start(out=cos_o[r0 : r0 + 128, :], in_=cos_strip[:])
                    nc.scalar.dma_start(out=ml_o[r0 : r0 + 128, :], in_=ml_strip[:])

                def mm_block(m, c):
                    """Phased start: all 4 k-steps of one (row, chunk) pair,
                    evicted immediately. Runs as soon as chunk c is prepped."""
                    nsz = NSZ[c]
                    ps = mm_psum.tile([128, 512], F32, tag="mm",
                                      name=f"pmm{m}_{c}{sfx}")
                    for k in range(KCH):
                        nc.tensor.matmul(
                            ps[:, :nsz],
                            lhsT=nfT[m][:, k * 128 : (k + 1) * 128],
                            rhs=nwT[c][:, k * nsz : (k + 1) * nsz],
                            start=(k == 0),
                            stop=(k == KCH - 1),
                        )
                    evict(m, c, ps)

                # --- prep + phased start -------------------------------
                # Loads stream on the Pool/SWDGE queue; rows m=0..2 are
                # matmul'd chunk-by-chunk as each nwT chunk completes, so PE
                # has real work while the ACT/DVE prep chains (the prep
                # bottleneck, ~0.85us/tile) run.
                fq_tiles[0] = feat_quad_load(0)
                wqA = w_load(0, 4, f"xwA{sfx}")
                wqB = w_load(4, 4, f"xwB{sfx}")
                wqC = w_load(8, 4, f"xwC{sfx}")
                fq_tiles[1] = feat_quad_load(1)
                wqD = w_load(12, 4, f"xwD{sfx}")
                # tail: tiles 16-18 (3x128 rows) + 19 (68 rows)
                wt5 = stage.tile([128, 4 * D], BF16, tag="xt4",
                                 name=f"xw5{sfx}")
                nc.gpsimd.dma_start(
                    out=wt5[:, : 3 * D].rearrange("p (s c) -> p s c", s=3),
                    in_=w[16 * 128 : 19 * 128, :].rearrange(
                        "(s p) c -> p s c", s=3
                    ),
                )
                nc.gpsimd.dma_start(
                    out=wt5[:68, 3 * D :], in_=w[19 * 128 : CLOC, :]
                )
                fq_tiles[2] = feat_quad_load(2)

                for b in (0, 1):
                    feat_prep_tile(fq_tiles[0][:, b * D : (b + 1) * D], b)
                for b in range(4):
                    wprep_tile(wqA[:, b * D : (b + 1) * D], b, 128)
                for m in range(M0):
                    new_strips(m)
                mm_block(0, 0)
                for b in (2, 3):
                    feat_prep_tile(fq_tiles[0][:, b * D : (b + 1) * D], b)
                for b in range(4):
                    wprep_tile(wqB[:, b * D : (b + 1) * D], 4 + b, 128)
                mm_block(1, 0)
                mm_block(0, 1)
                for b in range(4):
                    wprep_tile(wqC[:, b * D : (b + 1) * D], 8 + b, 128)
                mm_block(2, 0)
                mm_block(1, 1)
                mm_block(0, 2)
                for b in range(4):
                    feat_prep_tile(fq_tiles[1][:, b * D : (b + 1) * D], 4 + b)
                for b in range(4):
                    wprep_tile(wqD[:, b * D : (b + 1) * D], 12 + b, 128)
                mm_block(2, 1)
                mm_block(1, 2)
                mm_block(0, 3)
                for b in range(3):
                    wprep_tile(wt5[:, b * D : (b + 1) * D], 16 + b, 128)
                wprep_tile(wt5[:, 3 * D :], 19, 68)
                mm_block(2, 2)
                mm_block(1, 3)
                mm_block(0, 4)
                finish_m(0)
                feat_prep_tile(fq_tiles[2][:, 0:D], 8)
                mm_block(2, 3)
                mm_block(1, 4)
                finish_m(1)
                feat_prep_tile(fq_tiles[2][:, D : 2 * D], 9)
                mm_block(2, 4)
                finish_m(2)
                feat_prep_tile(fq_tiles[2][:, 2 * D : 3 * D], 10)

                # --- main loop: one m-strip per iteration ---
                for m in range(M0, MT):
                    ft = m + FT_AHEAD  # feat tile to prefetch/process
                    if ft < MT and ft % 4 == 0:
                        fq_tiles[ft // 4] = feat_quad_load(ft // 4)
                    new_strips(m)
                    # k-outer: the stationary lhsT block is loaded once per
                    # (m, k) and reused across the 5 n-chunks (ldweights=False
                    # on the reuse matmuls). One live PSUM bank per n-chunk.
                    pss = [
                        mm_psum.tile([128, 512], F32, tag="mm",
                                     name=f"mm{m}_{n}{sfx}")
                        for n in range(NT)
                    ]
                    for k in range(KCH):
                        for n in range(NT):
                            nsz = NSZ[n]
                            inst = nc.tensor.matmul(
                                pss[n][:, :nsz],
                                lhsT=nfT[m][:, k * 128 : (k + 1) * 128],
                                rhs=nwT[n][:, k * nsz : (k + 1) * nsz],
                                start=(k == 0),
                                stop=(k == KCH - 1),
                            )
                            if n > 0:
                                inst.ldweights = False
                    # cos evicts: banks are reused by row m+1 after ~0.2us per
                    # chunk, so split across ACT (n=0..2) and DVE (n=3..4).
                    for n in range(NT):
                        evict(m, n, pss[n])
                    finish_m(m)
                    # prefetched feat tile's prep goes AFTER this m's evicts:
                    # they are bank-critical on ACT/DVE's in-order queues,
                    # while the feat tile has 8 m-strips of slack.
                    if ft < MT:
                        feat_prep_tile(
                            fq_tiles[ft // 4][:, (ft % 4) * D : (ft % 4 + 1) * D],
                            ft,
                        )

            for rep in range(repeats):
                body(rep)

            if timing:
                # Fence: read back a sliver of each Internal output on both
                # output rings (FIFO per ring), keeping writes live vs
                # dead-store elimination and gating the token on the drain.
                tok = const_pool.tile([128, 4], F32, tag="tok")
                tokb = const_pool.tile([128, 4], BF16, tag="tokb")
                nc.sync.dma_start(out=tokb[:, :2], in_=cos_o[B - 128 :, :2])
                nc.scalar.dma_start(out=tokb[:, 2:4], in_=ml_o[B - 128 :, :2])
                nc.vector.tensor_copy(tok[:], tokb[:])
                nc.sync.dma_start(out=tok_o, in_=tok[:])

    nc.compile()
    return nc


def _purge_neff_cache():
    """The neuronxcc NEFF cache key does NOT cover the embedded BIR
    payload (verified: edited kernels cache-hit stale NEFFs compiled
    from different BIR). Purge it so this process always executes the
    NEFF compiled from THIS module."""
    import shutil

    shutil.rmtree("/root/.neuron-compile-cache", ignore_errors=True)


def _get_nc():
    if "nc" not in _NC_CACHE:
        _purge_neff_cache()
        _NC_CACHE["nc"] = _build_nc()
    return _NC_CACHE["nc"]


def make_in_maps(feat, weights):
    feat = np.ascontiguousarray(np.asarray(feat, dtype=np.float32))
    weights = np.ascontiguousarray(np.asarray(weights, dtype=np.float32))
    return [
        {"feat": feat, "w": weights[k * CLOC : (k + 1) * CLOC]}
        for k in range(NCORES)
    ]


def assemble(results, label):
    """Gather per-core column slices (bf16 -> f32) and apply the per-row
    label fixup."""
    cos = np.empty((B, C), np.float32)
    ml = np.empty((B, C), np.float32)
    for k in range(NCORES):
        cos[:, k * CLOC : (k + 1) * CLOC] = results[k]["cos_o"].astype(
            np.float32
        )
        ml[:, k * CLOC : (k + 1) * CLOC] = results[k]["ml_o"].astype(
            np.float32
        )
    idx = np.arange(B)
    lab = np.asarray(label).astype(np.int64)
    cil = cos[idx, lab]
    sin_il = np.sqrt(np.maximum(0.0, 1.0 - cil * cil)).astype(np.float32)
    hit = cil > THRESH
    ml[idx, lab] = np.where(
        hit,
        SCALE * (cil * COS_M - sin_il * SIN_M),
        SCALE * (cil + EXT_VAL),
    ).astype(np.float32)
    return cos, ml


def kernel(feat, label, weights):
    nc = _get_nc()
    in_maps = make_in_maps(feat, weights)
    res = run_bass_kernel_spmd(nc, in_maps, core_ids=list(range(NCORES)))
    return assemble(res.results, label)
